# revision 36
# baseline (speedup 1.0000x reference)
"""Trainium2 Bass kernel for nn_CaslsChineseAttnLoss (label-smoothed KLDiv loss).

Math (per flattened token n, vocab size V):
    weight row = off_n everywhere except src_n at the target column t_n, with
        off_n = sm_n * matric[forth_n, t_n] / (V-1),  src_n = 1 - V*off_n
    kl_n = (V-1)*off*ln(off) + src*ln(src) - off*S_n - (src-off)*logp_{n,t_n}
    where S_n = sum_v logp_{n,v} = sumx_n - V*lse_n, lse_n = ln(sum_v exp x_nv).
    loss = sum_n kl_n / sum_b (label_lengths_b + 1)

Sharding: data-parallel over the token dim N=4096 — 512 rows per core across
8 cores; the per-token confusion values matric[forth, t] and target logits
x[n, t_n] are gathered on the host during input sharding (the [N]-sized
gathered-values exchange the sharding hint describes) so the device streams
only dense data; each core emits a scalar partial and the host combines the
8 floats (an on-device AllReduce psum costs ~30us of cross-core skew-wait
for a tiny payload).

Device kernel per core: stream the [512, 8192] f32 shard through SBUF in
column chunks over the four 128-row tiles; ACT computes exp with accum (row
sum-exp), DVE reduce_sum computes row sums, both overlapping the HBM DMA.
The system is engine-bound, not HBM-bound (the DMA sustains 400+ GB/s while
each engine consumes ~1 col/cycle), so the chunk plan minimizes engine busy
time: wide 4096 chunks in the middle (low per-chunk overhead), small head
chunks so the engines start during the DMA ramp, a decreasing tail so the
final chunk's compute is short, and one mid-stream chunk's row-sum split
between ACT (Copy+accum) and DVE to balance their loads.  All loads ride
the sync HWDGE ring — the scalar ring starves behind the saturated stream
queue, and a [128,1] DMA store is 128 sub-512B HBM RMWs (~7us), hence the
matmul partition-reduce to a single 4-byte store.  Per-tile partials
collapse mid-stream; only the last tile's reduce, one batched [128,4] Ln and
a short vector chain trail the final DMA byte.  exp is computed without max
subtraction — inputs are unit-normal logits, so sum-exp stays in fp32 range.
"""

import math

import numpy as np

import concourse.tile as tile
from concourse import bacc, mybir
from concourse import bass_utils
from concourse.hw_specs import get_activation_tables

ALPHA = 0.1
B, T, V = 8, 512, 8192
N = B * T                 # 4096 flattened tokens
N_CORES = 8
NLOC = N // N_CORES       # 512 rows per core
P = 128                   # partitions
NT = NLOC // P            # 4 row tiles per core
F32 = mybir.dt.float32
I32 = mybir.dt.int32

# chunk plan: (row_tile, col_start, width).  The stream is engine-bound, not
# HBM-bound (DMA sustains 400+ GB/s while ACT/DVE consume ~1 col/cycle), so
# the plan minimizes engine busy time: few wide chunks in the middle (low
# per-chunk overhead), a small first chunk so the engines start early, and a
# decreasing tail sized so each chunk's engine time stays under the next
# chunk's DMA time (w_{k+1} >= 0.57*w_k + 390 keeps ACT tracking).
_TILE_WIDTHS = [
    [1024, 3072, 4096],
    [4096, 4096],
    [4096, 4096],
    [4096, 3840, 256],
]
# chunk whose row-sum is split: DVE reduces cols [SPLIT_W:], ACT sums cols
# [:SPLIT_W] via Copy+accum — balances ACT vs DVE total busy.  Mid-stream
# placement: early placement makes the split's ACT-copy a pool-slot
# consumer whose lateness (cascaded from the DMA ramp) head-blocks the
# sync FIFO for the tail chunks (measured 8us stall).
SPLIT_CI = 5
SPLIT_W = 2560
CHUNK_PLAN = []
for _j, _ws in enumerate(_TILE_WIDTHS):
    _c = 0
    for _w in _ws:
        CHUNK_PLAN.append((_j, _c, _w))
        _c += _w
    assert _c == V
NCHUNKS = len(CHUNK_PLAN)
# sumexp part col = chunk index; sumx part cols = one per chunk plus an
# extra col right after the split chunk's (keeps every tile's cols contiguous)
SUMX_COL = []
_pc = 0
for _ci in range(NCHUNKS):
    SUMX_COL.append(_pc)
    _pc += 2 if _ci == SPLIT_CI else 1
NXPARTS = _pc
TILE_ECOLS = []   # sumexp parts range per tile
TILE_XCOLS = []   # sumx parts range per tile
_pe = 0
for _j, _ws in enumerate(_TILE_WIDTHS):
    _n = len(_ws)
    TILE_ECOLS.append((_pe, _pe + _n))
    _x0 = SUMX_COL[_pe]
    _x1 = SUMX_COL[_pe + _n] if _pe + _n < NCHUNKS else NXPARTS
    TILE_XCOLS.append((_x0, _x1))
    _pe += _n

_CACHE = {}


def _build():
    if "nc" in _CACHE:
        return _CACHE["nc"]

    nc = bacc.Bacc("TRN2", target_bir_lowering=False, debug=False,
                   num_devices=N_CORES)

    x_d = nc.dram_tensor("x", [NLOC, V], F32, kind="ExternalInput")
    # ns | xt | lenrow packed in one tensor: one DMA with 48B partition
    # lines instead of three with 16B lines — 3x fewer tiny descriptors in
    # the stream FIFO (each 16B-line load cost ~0.5us of stream time)
    side_d = nc.dram_tensor("side", [P, 3 * NT], F32, kind="ExternalInput")
    out_d = nc.dram_tensor("out", [1, 1], F32, kind="ExternalOutput")

    AF = mybir.ActivationFunctionType
    AX = mybir.AxisListType.X
    MUL = mybir.AluOpType.mult
    ADD = mybir.AluOpType.add

    with tile.TileContext(nc) as tc:
        with tc.tile_pool(name="xchunk", bufs=9) as xpool, \
             tc.tile_pool(name="scratch", bufs=2) as spool, \
             tc.tile_pool(name="stats", bufs=1) as stats, \
             tc.tile_pool(name="psum", bufs=1, space="PSUM") as psump:

            # pre-load the ACT table set that has BOTH exp and ln, so the
            # greedy per-func table pass inserts zero switches
            tabs = list(get_activation_tables(nc.m.arch).keys())
            nc.scalar.add_instruction(mybir.InstLoadActFuncSet(
                name=nc.get_next_instruction_name(),
                act_func_set_id=tabs.index("natural_log_exp_and_others"),
                ins=[], outs=[]))

            sumexp_parts = stats.tile([P, NCHUNKS], F32)
            sumx_parts = stats.tile([P, NXPARTS], F32)
            side = stats.tile([P, 3 * NT], F32)
            ns = side[:, 0:NT]
            xt = side[:, NT:2 * NT]
            lenr = side[:, 2 * NT:3 * NT]
            eps = stats.tile([P, 1], F32)
            nc.vector.memset(eps[:], 1e-30)
            ones = stats.tile([P, 1], F32)
            nc.vector.memset(ones[:], 1.0)
            invlen = stats.tile([P, NT], F32)
            e1 = stats.tile([P, NT], F32)
            smc = stats.tile([P, NT], F32)

            def emit_side_loads():
                # on the sync ring behind chunks 0/1: queued in the stream's
                # own FIFO it completes promptly (~11us); on the scalar ring
                # (queue 10) it starves behind the saturated stream queue
                # and completes 15-25us late, stalling every consumer
                nc.sync.dma_start(side[:], side_d.ap())

            def emit_side_chain():
                # sm-coefficient chain; emitted after chunk 2's ops so the
                # scheduler orders the first chunk exps/reduces ahead of it
                # in the ACT/DVE queues (emitted first, it head-blocks both
                # engines on the side-load semaphores)
                nc.vector.reciprocal(invlen[:], lenr[:])
                nc.scalar.activation(e1[:], invlen[:], AF.Exp,
                                     scale=math.log(1.0 - ALPHA))
                nc.vector.tensor_scalar(smc[:], e1[:],
                                        -1.0 / (V - 1), 1.0 / (V - 1),
                                        op0=MUL, op1=ADD)

            # per-row constants, folded so the per-tile tail is minimal:
            #   kl_row = c1p - off*sumx + c3*lse        (proof: expand
            #   (V-1)xlogy(off) + xlogy(src) - off*(sumx - V*lse)
            #     - (src-off)*(xt - lse)  with c2 = src-off)
            off = stats.tile([P, NT], F32)
            src = stats.tile([P, NT], F32)
            lnoff = stats.tile([P, NT], F32)
            lnsrc = stats.tile([P, NT], F32)
            c2 = stats.tile([P, NT], F32)
            c3 = stats.tile([P, NT], F32)
            c1p = stats.tile([P, NT], F32)
            tmp = stats.tile([P, NT], F32)

            def emit_const_stats(pin_after):
                i0 = nc.vector.tensor_mul(off[:], smc[:], ns[:])
                # pin the chain root behind an early chunk op so the
                # scheduler can't hoist it ahead of the stream start and
                # head-block the engine queues on the gather semaphore
                tile.add_dep_helper(i0.ins, pin_after.ins, False,
                                    "const-stats after stream start")
                nc.vector.tensor_scalar(src[:], off[:], -float(V), 1.0,
                                        op0=MUL, op1=ADD)
                nc.scalar.activation(lnoff[:], off[:], AF.Ln, bias=eps[:])
                nc.scalar.activation(lnsrc[:], src[:], AF.Ln)
                nc.vector.tensor_mul(c1p[:], off[:], lnoff[:])
                nc.vector.tensor_scalar(c1p[:], c1p[:], float(V - 1), None,
                                        op0=MUL)
                nc.vector.tensor_mul(tmp[:], src[:], lnsrc[:])
                nc.vector.tensor_add(c1p[:], c1p[:], tmp[:])
                nc.vector.tensor_sub(c2[:], src[:], off[:])
                nc.vector.tensor_scalar(c3[:], off[:], float(V), None,
                                        op0=MUL)
                nc.vector.tensor_add(c3[:], c3[:], c2[:])
                nc.vector.tensor_mul(tmp[:], c2[:], xt[:])
                nc.vector.tensor_sub(c1p[:], c1p[:], tmp[:])

            sumexp = stats.tile([P, NT], F32)
            sumx = stats.tile([P, NT], F32)
            lse = stats.tile([P, NT], F32)
            acc = stats.tile([P, 1], F32)
            t1 = stats.tile([P, NT], F32)
            t2 = stats.tile([P, NT], F32)

            def emit_tile_reduce(j):
                # collapse row tile j's chunk partials to [P,1] sums;
                # sumexp first so the last tile's Ln starts one DVE op sooner
                e0, e1c = TILE_ECOLS[j]
                x0, x1 = TILE_XCOLS[j]
                nc.vector.reduce_sum(sumexp[:, j:j + 1],
                                     sumexp_parts[:, e0:e1c], axis=AX)
                nc.vector.reduce_sum(sumx[:, j:j + 1],
                                     sumx_parts[:, x0:x1], axis=AX)

            # streaming pass: per chunk, ACT exp+accum and DVE row-sum.
            # Side work is staged a few chunks in so the scheduler orders
            # the stream ops first; tile partial-reductions lag one tile.
            for ci, (j, c0, w) in enumerate(CHUNK_PLAN):
                xtile = xpool.tile([P, w], F32, tag="xchunk")
                # c2 rides the otherwise-idle gpsimd SWDGE queue: a second
                # DMA queue draining concurrently with the sync ring during
                # the ramp (the ramp is per-queue-FIFO-depth limited), with
                # no sequencer contention — Q7 only runs this one desc-gen
                ring = nc.gpsimd if ci == 2 else nc.sync
                ring.dma_start(
                    xtile[:], x_d.ap()[j * P:(j + 1) * P, c0:c0 + w])
                sc = spool.tile([P, w], F32, tag="scratch")
                nc.scalar.activation(
                    sc[:], xtile[:], AF.Exp,
                    accum_out=sumexp_parts[:, ci:ci + 1])
                xc = SUMX_COL[ci]
                if ci == SPLIT_CI:
                    # balance engines: ACT row-sums the first SPLIT_W cols
                    # (Copy+accum), DVE reduces the rest
                    sc2 = spool.tile([P, SPLIT_W], F32, tag="scratch")
                    nc.scalar.activation(
                        sc2[:], xtile[:, 0:SPLIT_W], AF.Copy,
                        accum_out=sumx_parts[:, xc + 1:xc + 2])
                    red = nc.vector.reduce_sum(
                        sumx_parts[:, xc:xc + 1], xtile[:, SPLIT_W:w],
                        axis=AX)
                else:
                    red = nc.vector.reduce_sum(
                        sumx_parts[:, xc:xc + 1], xtile[:], axis=AX)
                if ci == 1:
                    emit_side_loads()
                elif ci == 2:
                    emit_side_chain()
                elif ci == 3:
                    emit_const_stats(pin_after=red)
                elif ci == 5:
                    emit_tile_reduce(0)
                elif ci == 7:
                    emit_tile_reduce(1)
                elif ci == 9:
                    emit_tile_reduce(2)
            emit_tile_reduce(NT - 1)

            # batched finalize: one [P,NT]-wide chain instead of per-tile
            # [P,1] ops — fewer DVE instructions, and only mul+add+rowsum
            # trail the last tile's Ln
            nc.scalar.activation(lse[:], sumexp[:], AF.Ln)
            nc.vector.tensor_mul(t1[:], off[:], sumx[:])
            nc.vector.tensor_sub(t1[:], c1p[:], t1[:])
            nc.vector.tensor_mul(t2[:], c3[:], lse[:])
            nc.vector.tensor_add(t1[:], t1[:], t2[:])
            nc.vector.reduce_sum(acc[:], t1[:], axis=AX)

            # partition-reduce via matmul into PSUM, then a single 4-byte
            # store: a [128,1] store is 128 sub-512B HBM RMWs whose last
            # sem-inc lands ~7us after the data (measured) — far worse than
            # LDWEIGHTS+MATMUL+copy (~0.5us)
            tot_psum = psump.tile([1, 1], F32)
            nc.tensor.matmul(tot_psum[:], lhsT=acc[:], rhs=ones[:],
                             start=True, stop=True)
            tot = stats.tile([1, 1], F32)
            nc.vector.tensor_copy(tot[:], tot_psum[:])
            nc.sync.dma_start(out_d.ap(), tot[:])

    nc.compile()
    _CACHE["nc"] = nc
    return nc


def _prep_in_maps(inputs, matric, targets, label_lengths):
    x = np.ascontiguousarray(np.asarray(inputs, dtype=np.float32)).reshape(N, V)
    t = np.asarray(targets).reshape(-1).astype(np.int64)
    lab = np.asarray(label_lengths).reshape(-1).astype(np.int64)
    mat = np.asarray(matric, dtype=np.float32).reshape(V * V)

    eos = (t == 1)
    prev = np.roll(t, 1)
    is_start = np.roll(eos, 1)
    is_start[0] = True
    forth = np.where(is_start, N - 1, prev)
    seg = np.cumsum(eos.astype(np.int64)) - eos.astype(np.int64)
    length = lab + 1
    # jax gather clamps out-of-range indices; mirror that
    len_row = length[np.clip(seg, 0, B - 1)].astype(np.float32)
    t_cl = np.clip(t, 0, V - 1)
    # host-side gathers of the [N] per-token values (sharding prep)
    need_sm = mat[np.clip(forth, 0, V - 1) * V + t_cl].astype(np.float32)
    x_tgt = x[np.arange(N), t_cl].astype(np.float32)
    lensum = np.float32(length.sum())

    in_maps = []
    for c in range(N_CORES):
        sl = slice(c * NLOC, (c + 1) * NLOC)
        side = np.concatenate([
            need_sm[sl].reshape(NT, P).T,
            x_tgt[sl].reshape(NT, P).T,
            len_row[sl].reshape(NT, P).T,
        ], axis=1)
        in_maps.append({
            "x": np.ascontiguousarray(x[sl]),
            "side": np.ascontiguousarray(side),
        })
    return in_maps, lensum


def run(inputs, matric, targets, label_lengths, trace=False):
    nc = _build()
    in_maps, lensum = _prep_in_maps(inputs, matric, targets, label_lengths)
    if trace:
        _install_ntff_hook()
    res = bass_utils.run_bass_kernel_spmd(
        nc, in_maps, core_ids=list(range(N_CORES)), trace=trace)
    partials = np.array(
        [res.results[c]["out"][0, 0] for c in range(N_CORES)], dtype=np.float32)
    out = np.float32(partials.sum(dtype=np.float32) / lensum)
    return np.asarray(out), res


def kernel(inputs, matric, targets, label_lengths):
    out, _ = run(inputs, matric, targets, label_lengths, trace=False)
    return out


def _install_ntff_hook():
    """bass_utils expects antenv.axon_hooks for NTFF tracing under axon; the
    agent image lacks it, so recreate the ctypes shim inline."""
    import contextlib
    import ctypes
    import sys
    import types

    if "antenv.axon_hooks" in sys.modules:
        return
    so_path = "/opt/axon/libaxon_pjrt.so"
    try:
        lib = ctypes.CDLL(so_path)
    except OSError:
        return
    if not hasattr(lib, "axon_start_nrt_profile"):
        return
    lib.axon_start_nrt_profile.argtypes = [
        ctypes.POINTER(ctypes.c_int64), ctypes.c_size_t]
    lib.axon_start_nrt_profile.restype = ctypes.c_int64
    lib.axon_stop_nrt_profile.argtypes = [ctypes.c_char_p]
    lib.axon_stop_nrt_profile.restype = ctypes.c_int64

    @contextlib.contextmanager
    def _hook(output_dir, device_ids):
        import jax
        jax.devices()
        ids = list(device_ids) if device_ids else []
        arr = (ctypes.c_int64 * len(ids))(*ids)
        rc = lib.axon_start_nrt_profile(arr, len(ids))
        if rc != 0:
            raise RuntimeError(f"axon_start_nrt_profile rc={rc}")
        try:
            yield
        finally:
            n = lib.axon_stop_nrt_profile(str(output_dir).encode())
            if n < 0:
                raise RuntimeError(f"axon_stop_nrt_profile rc={n}")

    mod = types.ModuleType("antenv.axon_hooks")
    mod.get_axon_ntff_profile_hook = lambda: _hook
    mod.set_axon_ntff_profile_hook = lambda h: None
    sys.modules["antenv.axon_hooks"] = mod


# revision 37
# speedup vs baseline: 1.1102x; 1.1102x over previous
"""Trainium2 Bass kernel for nn_CaslsChineseAttnLoss (label-smoothed KLDiv loss).

Math (per flattened token n, vocab size V):
    weight row = off_n everywhere except src_n at the target column t_n, with
        off_n = sm_n * matric[forth_n, t_n] / (V-1),  src_n = 1 - V*off_n
    kl_n = (V-1)*off*ln(off) + src*ln(src) - off*S_n - (src-off)*logp_{n,t_n}
    where S_n = sum_v logp_{n,v} = sumx_n - V*lse_n, lse_n = ln(sum_v exp x_nv).
    loss = sum_n kl_n / sum_b (label_lengths_b + 1)

Sharding: data-parallel over the token dim N=4096 — 512 rows per core across
8 cores; the per-token confusion values matric[forth, t] and target logits
x[n, t_n] are gathered on the host during input sharding (the [N]-sized
gathered-values exchange the sharding hint describes) so the device streams
only dense data; each core emits a scalar partial and the host combines the
8 floats (an on-device AllReduce psum costs ~30us of cross-core skew-wait
for a tiny payload).

Device kernel per core: stream the [512, 8192] f32 shard through SBUF in
column chunks over the four 128-row tiles; ACT computes exp with accum (row
sum-exp), DVE reduce_sum computes row sums, both overlapping the HBM DMA.
The system is engine-bound, not HBM-bound (the DMA sustains 400+ GB/s while
each engine consumes ~1 col/cycle), so the chunk plan minimizes engine busy
time: wide 4096 chunks in the middle (low per-chunk overhead), small head
chunks so the engines start during the DMA ramp, a decreasing tail so the
final chunk's compute is short, and one mid-stream chunk's row-sum split
between ACT (Copy+accum) and DVE to balance their loads.  All loads ride
the sync HWDGE ring — the scalar ring starves behind the saturated stream
queue, and a [128,1] DMA store is 128 sub-512B HBM RMWs (~7us), hence the
matmul partition-reduce to a single 4-byte store.  Per-tile partials
collapse mid-stream; only the last tile's reduce, one batched [128,4] Ln and
a short vector chain trail the final DMA byte.  exp is computed without max
subtraction — inputs are unit-normal logits, so sum-exp stays in fp32 range.
"""

import math

import numpy as np

import concourse.tile as tile
from concourse import bacc, mybir
from concourse import bass_utils
from concourse.hw_specs import get_activation_tables

ALPHA = 0.1
B, T, V = 8, 512, 8192
N = B * T                 # 4096 flattened tokens
N_CORES = 8
NLOC = N // N_CORES       # 512 rows per core
P = 128                   # partitions
NT = NLOC // P            # 4 row tiles per core
F32 = mybir.dt.float32
I32 = mybir.dt.int32

# chunk plan: (row_tile, col_start, width).  The stream is engine-bound, not
# HBM-bound (DMA sustains 400+ GB/s while ACT/DVE consume ~1 col/cycle), so
# the plan minimizes engine busy time: few wide chunks in the middle (low
# per-chunk overhead), a small first chunk so the engines start early, and a
# decreasing tail sized so each chunk's engine time stays under the next
# chunk's DMA time (w_{k+1} >= 0.57*w_k + 390 keeps ACT tracking).
_TILE_WIDTHS = [
    [1024, 3072, 4096],
    [4096, 4096],
    [4096, 4096],
    [4096, 3840, 256],
]
# chunk whose row-sum is split: DVE reduces cols [SPLIT_W:], ACT sums cols
# [:SPLIT_W] via Copy+accum — balances ACT vs DVE total busy.  Mid-stream
# placement: early placement makes the split's ACT-copy a pool-slot
# consumer whose lateness (cascaded from the DMA ramp) head-blocks the
# sync FIFO for the tail chunks (measured 8us stall).
SPLIT_CI = 5
SPLIT_W = 2560
CHUNK_PLAN = []
for _j, _ws in enumerate(_TILE_WIDTHS):
    _c = 0
    for _w in _ws:
        CHUNK_PLAN.append((_j, _c, _w))
        _c += _w
    assert _c == V
NCHUNKS = len(CHUNK_PLAN)
# sumexp part col = chunk index; sumx part cols = one per chunk plus an
# extra col right after the split chunk's (keeps every tile's cols contiguous)
SUMX_COL = []
_pc = 0
for _ci in range(NCHUNKS):
    SUMX_COL.append(_pc)
    _pc += 2 if _ci == SPLIT_CI else 1
NXPARTS = _pc
TILE_ECOLS = []   # sumexp parts range per tile
TILE_XCOLS = []   # sumx parts range per tile
_pe = 0
for _j, _ws in enumerate(_TILE_WIDTHS):
    _n = len(_ws)
    TILE_ECOLS.append((_pe, _pe + _n))
    _x0 = SUMX_COL[_pe]
    _x1 = SUMX_COL[_pe + _n] if _pe + _n < NCHUNKS else NXPARTS
    TILE_XCOLS.append((_x0, _x1))
    _pe += _n

_CACHE = {}


def _build():
    if "nc" in _CACHE:
        return _CACHE["nc"]

    nc = bacc.Bacc("TRN2", target_bir_lowering=False, debug=False,
                   num_devices=N_CORES)

    x_d = nc.dram_tensor("x", [NLOC, V], F32, kind="ExternalInput")
    # ns | xt | lenrow packed in one tensor: one DMA with 48B partition
    # lines instead of three with 16B lines — 3x fewer tiny descriptors in
    # the stream FIFO (each 16B-line load cost ~0.5us of stream time)
    side_d = nc.dram_tensor("side", [P, 3 * NT], F32, kind="ExternalInput")
    out_d = nc.dram_tensor("out", [1, 1], F32, kind="ExternalOutput")

    AF = mybir.ActivationFunctionType
    AX = mybir.AxisListType.X
    MUL = mybir.AluOpType.mult
    ADD = mybir.AluOpType.add

    with tile.TileContext(nc) as tc:
        with tc.tile_pool(name="xchunk", bufs=9) as xpool, \
             tc.tile_pool(name="scratch", bufs=2) as spool, \
             tc.tile_pool(name="stats", bufs=1) as stats, \
             tc.tile_pool(name="psum", bufs=1, space="PSUM") as psump:

            # pre-load the ACT table set that has BOTH exp and ln, so the
            # greedy per-func table pass inserts zero switches
            tabs = list(get_activation_tables(nc.m.arch).keys())
            nc.scalar.add_instruction(mybir.InstLoadActFuncSet(
                name=nc.get_next_instruction_name(),
                act_func_set_id=tabs.index("natural_log_exp_and_others"),
                ins=[], outs=[]))

            sumexp_parts = stats.tile([P, NCHUNKS], F32)
            sumx_parts = stats.tile([P, NXPARTS], F32)
            side = stats.tile([P, 3 * NT], F32)
            ns = side[:, 0:NT]
            xt = side[:, NT:2 * NT]
            lenr = side[:, 2 * NT:3 * NT]
            eps = stats.tile([P, 1], F32)
            nc.vector.memset(eps[:], 1e-30)
            ones = stats.tile([P, 1], F32)
            nc.vector.memset(ones[:], 1.0)
            invlen = stats.tile([P, NT], F32)
            e1 = stats.tile([P, NT], F32)
            smc = stats.tile([P, NT], F32)

            def emit_side_loads():
                # on the sync ring behind chunks 0/1: queued in the stream's
                # own FIFO it completes promptly (~11us); on the scalar ring
                # (queue 10) it starves behind the saturated stream queue
                # and completes 15-25us late, stalling every consumer
                nc.sync.dma_start(side[:], side_d.ap())

            def emit_side_chain():
                # sm-coefficient chain; emitted after chunk 2's ops so the
                # scheduler orders the first chunk exps/reduces ahead of it
                # in the ACT/DVE queues (emitted first, it head-blocks both
                # engines on the side-load semaphores)
                nc.vector.reciprocal(invlen[:], lenr[:])
                nc.scalar.activation(e1[:], invlen[:], AF.Exp,
                                     scale=math.log(1.0 - ALPHA))
                nc.vector.tensor_scalar(smc[:], e1[:],
                                        -1.0 / (V - 1), 1.0 / (V - 1),
                                        op0=MUL, op1=ADD)

            # per-row constants, folded so the per-tile tail is minimal:
            #   kl_row = c1p - off*sumx + c3*lse        (proof: expand
            #   (V-1)xlogy(off) + xlogy(src) - off*(sumx - V*lse)
            #     - (src-off)*(xt - lse)  with c2 = src-off)
            off = stats.tile([P, NT], F32)
            src = stats.tile([P, NT], F32)
            lnoff = stats.tile([P, NT], F32)
            lnsrc = stats.tile([P, NT], F32)
            c2 = stats.tile([P, NT], F32)
            c3 = stats.tile([P, NT], F32)
            c1p = stats.tile([P, NT], F32)
            tmp = stats.tile([P, NT], F32)

            def emit_const_stats(pin_after):
                i0 = nc.vector.tensor_mul(off[:], smc[:], ns[:])
                # pin the chain root behind an early chunk op so the
                # scheduler can't hoist it ahead of the stream start and
                # head-block the engine queues on the gather semaphore
                tile.add_dep_helper(i0.ins, pin_after.ins, False,
                                    "const-stats after stream start")
                nc.vector.tensor_scalar(src[:], off[:], -float(V), 1.0,
                                        op0=MUL, op1=ADD)
                nc.scalar.activation(lnoff[:], off[:], AF.Ln, bias=eps[:])
                nc.scalar.activation(lnsrc[:], src[:], AF.Ln)
                nc.vector.tensor_mul(c1p[:], off[:], lnoff[:])
                nc.vector.tensor_scalar(c1p[:], c1p[:], float(V - 1), None,
                                        op0=MUL)
                nc.vector.tensor_mul(tmp[:], src[:], lnsrc[:])
                nc.vector.tensor_add(c1p[:], c1p[:], tmp[:])
                nc.vector.tensor_sub(c2[:], src[:], off[:])
                nc.vector.tensor_scalar(c3[:], off[:], float(V), None,
                                        op0=MUL)
                nc.vector.tensor_add(c3[:], c3[:], c2[:])
                nc.vector.tensor_mul(tmp[:], c2[:], xt[:])
                nc.vector.tensor_sub(c1p[:], c1p[:], tmp[:])

            sumexp = stats.tile([P, NT], F32)
            sumx = stats.tile([P, NT], F32)
            lse = stats.tile([P, NT], F32)
            acc = stats.tile([P, 1], F32)
            t1 = stats.tile([P, NT], F32)
            t2 = stats.tile([P, NT], F32)

            def emit_tile_reduce(j):
                # collapse row tile j's chunk partials to [P,1] sums;
                # sumexp first so the last tile's Ln starts one DVE op sooner
                e0, e1c = TILE_ECOLS[j]
                x0, x1 = TILE_XCOLS[j]
                nc.vector.reduce_sum(sumexp[:, j:j + 1],
                                     sumexp_parts[:, e0:e1c], axis=AX)
                nc.vector.reduce_sum(sumx[:, j:j + 1],
                                     sumx_parts[:, x0:x1], axis=AX)

            # streaming pass: per chunk, ACT exp+accum and DVE row-sum.
            # Side work is staged a few chunks in so the scheduler orders
            # the stream ops first; tile partial-reductions lag one tile.
            for ci, (j, c0, w) in enumerate(CHUNK_PLAN):
                xtile = xpool.tile([P, w], F32, tag="xchunk")
                nc.sync.dma_start(
                    xtile[:], x_d.ap()[j * P:(j + 1) * P, c0:c0 + w])
                sc = spool.tile([P, w], F32, tag="scratch")
                nc.scalar.activation(
                    sc[:], xtile[:], AF.Exp,
                    accum_out=sumexp_parts[:, ci:ci + 1])
                xc = SUMX_COL[ci]
                if ci == SPLIT_CI:
                    # balance engines: ACT row-sums the first SPLIT_W cols
                    # (Copy+accum), DVE reduces the rest
                    sc2 = spool.tile([P, SPLIT_W], F32, tag="scratch")
                    nc.scalar.activation(
                        sc2[:], xtile[:, 0:SPLIT_W], AF.Copy,
                        accum_out=sumx_parts[:, xc + 1:xc + 2])
                    red = nc.vector.reduce_sum(
                        sumx_parts[:, xc:xc + 1], xtile[:, SPLIT_W:w],
                        axis=AX)
                else:
                    red = nc.vector.reduce_sum(
                        sumx_parts[:, xc:xc + 1], xtile[:], axis=AX)
                if ci == 1:
                    emit_side_loads()
                elif ci == 2:
                    emit_side_chain()
                elif ci == 3:
                    emit_const_stats(pin_after=red)
                elif ci == 5:
                    emit_tile_reduce(0)
                elif ci == 7:
                    emit_tile_reduce(1)
                elif ci == 9:
                    emit_tile_reduce(2)
            emit_tile_reduce(NT - 1)

            # batched finalize: one [P,NT]-wide chain instead of per-tile
            # [P,1] ops — fewer DVE instructions, and only mul+add+rowsum
            # trail the last tile's Ln
            nc.scalar.activation(lse[:], sumexp[:], AF.Ln)
            nc.vector.tensor_mul(t1[:], off[:], sumx[:])
            nc.vector.tensor_sub(t1[:], c1p[:], t1[:])
            nc.vector.tensor_mul(t2[:], c3[:], lse[:])
            nc.vector.tensor_add(t1[:], t1[:], t2[:])
            nc.vector.reduce_sum(acc[:], t1[:], axis=AX)

            # partition-reduce via matmul into PSUM, then a single 4-byte
            # store: a [128,1] store is 128 sub-512B HBM RMWs whose last
            # sem-inc lands ~7us after the data (measured) — far worse than
            # LDWEIGHTS+MATMUL+copy (~0.5us)
            tot_psum = psump.tile([1, 1], F32)
            nc.tensor.matmul(tot_psum[:], lhsT=acc[:], rhs=ones[:],
                             start=True, stop=True)
            tot = stats.tile([1, 1], F32)
            nc.vector.tensor_copy(tot[:], tot_psum[:])
            nc.sync.dma_start(out_d.ap(), tot[:])

    nc.compile()
    _CACHE["nc"] = nc
    return nc


def _prep_in_maps(inputs, matric, targets, label_lengths):
    x = np.ascontiguousarray(np.asarray(inputs, dtype=np.float32)).reshape(N, V)
    t = np.asarray(targets).reshape(-1).astype(np.int64)
    lab = np.asarray(label_lengths).reshape(-1).astype(np.int64)
    mat = np.asarray(matric, dtype=np.float32).reshape(V * V)

    eos = (t == 1)
    prev = np.roll(t, 1)
    is_start = np.roll(eos, 1)
    is_start[0] = True
    forth = np.where(is_start, N - 1, prev)
    seg = np.cumsum(eos.astype(np.int64)) - eos.astype(np.int64)
    length = lab + 1
    # jax gather clamps out-of-range indices; mirror that
    len_row = length[np.clip(seg, 0, B - 1)].astype(np.float32)
    t_cl = np.clip(t, 0, V - 1)
    # host-side gathers of the [N] per-token values (sharding prep)
    need_sm = mat[np.clip(forth, 0, V - 1) * V + t_cl].astype(np.float32)
    x_tgt = x[np.arange(N), t_cl].astype(np.float32)
    lensum = np.float32(length.sum())

    in_maps = []
    for c in range(N_CORES):
        sl = slice(c * NLOC, (c + 1) * NLOC)
        side = np.concatenate([
            need_sm[sl].reshape(NT, P).T,
            x_tgt[sl].reshape(NT, P).T,
            len_row[sl].reshape(NT, P).T,
        ], axis=1)
        in_maps.append({
            "x": np.ascontiguousarray(x[sl]),
            "side": np.ascontiguousarray(side),
        })
    return in_maps, lensum


def run(inputs, matric, targets, label_lengths, trace=False):
    nc = _build()
    in_maps, lensum = _prep_in_maps(inputs, matric, targets, label_lengths)
    if trace:
        _install_ntff_hook()
    res = bass_utils.run_bass_kernel_spmd(
        nc, in_maps, core_ids=list(range(N_CORES)), trace=trace)
    partials = np.array(
        [res.results[c]["out"][0, 0] for c in range(N_CORES)], dtype=np.float32)
    out = np.float32(partials.sum(dtype=np.float32) / lensum)
    return np.asarray(out), res


def kernel(inputs, matric, targets, label_lengths):
    out, _ = run(inputs, matric, targets, label_lengths, trace=False)
    return out


def _install_ntff_hook():
    """bass_utils expects antenv.axon_hooks for NTFF tracing under axon; the
    agent image lacks it, so recreate the ctypes shim inline."""
    import contextlib
    import ctypes
    import sys
    import types

    if "antenv.axon_hooks" in sys.modules:
        return
    so_path = "/opt/axon/libaxon_pjrt.so"
    try:
        lib = ctypes.CDLL(so_path)
    except OSError:
        return
    if not hasattr(lib, "axon_start_nrt_profile"):
        return
    lib.axon_start_nrt_profile.argtypes = [
        ctypes.POINTER(ctypes.c_int64), ctypes.c_size_t]
    lib.axon_start_nrt_profile.restype = ctypes.c_int64
    lib.axon_stop_nrt_profile.argtypes = [ctypes.c_char_p]
    lib.axon_stop_nrt_profile.restype = ctypes.c_int64

    @contextlib.contextmanager
    def _hook(output_dir, device_ids):
        import jax
        jax.devices()
        ids = list(device_ids) if device_ids else []
        arr = (ctypes.c_int64 * len(ids))(*ids)
        rc = lib.axon_start_nrt_profile(arr, len(ids))
        if rc != 0:
            raise RuntimeError(f"axon_start_nrt_profile rc={rc}")
        try:
            yield
        finally:
            n = lib.axon_stop_nrt_profile(str(output_dir).encode())
            if n < 0:
                raise RuntimeError(f"axon_stop_nrt_profile rc={n}")

    mod = types.ModuleType("antenv.axon_hooks")
    mod.get_axon_ntff_profile_hook = lambda: _hook
    mod.set_axon_ntff_profile_hook = lambda h: None
    sys.modules["antenv.axon_hooks"] = mod
